# revision 29
# baseline (speedup 1.0000x reference)
"""Trainium2 Bass kernel for nn_MetricLoss (pairwise metric loss, B=8192 D=128 k=4).

  d2[i,j] = sq_i + sq_j - 2*x_i.x_j
  loss_homo  = sum_{same group, i!=j} d2 / 24576
  loss_heter = sum_{g_i < g_j} relu(1 - d2) / 33538048

Circular half-window sharding over 8 NeuronCores: the 8192 rows form 64
blocks of 128.  Core p owns anchor blocks 8p..8p+7.  Anchor tile t
processes column blocks t+1..t+31 (distance 1..31, each cross-block
unordered pair exactly once) plus an additive distance-32 pass (each
dist-32 pair appears twice globally, so it enters the heter sum at half
weight).

All per-pair affine terms ride inside a single fp8e4 DoubleRow matmul
(0.5 cycles/row) by packing extra contraction rows:
  rows 0..63   x-halves (two k-tiles = 128 data dims)
  row 64       (b_n, 1) moving x (1, c_m) stationary -> + b_n + c_m
  rows 65..96  -128 * same-local-group rank-1 indicators (corr pass only)
so PSUM = G + (1 - sq_n)/2 - sq_m/2 = (1 - d2)/2 and the pointwise pass
is a bare relu+accumulate done in place on PSUM, split across ScalarE
and VectorE (GPSIMD cannot touch PSUM on TRN2; DMA cannot read PSUM).

Homo loss via the diagonal Gram blocks: homo_sum = sum (6I - 2*maskh) . G
over the 64 diag blocks (= 6*sum sq - 2*sum_maskh G), one masked
multiply-accumulate per core.
"""
import sys

sys.path.insert(0, "/opt/trn_rl_repo")

import numpy as np
import ml_dtypes
import concourse.bacc as bacc
import concourse.tile as tile
import concourse.mybir as mybir
from concourse import bass_utils
from contextlib import ExitStack

F32 = mybir.dt.float32
FP8 = mybir.dt.float8e4
DR = mybir.MatmulPerfMode.DoubleRow
F8NP = ml_dtypes.float8_e4m3

B, D, K = 8192, 128, 4
NCORES = 8
RPC = B // NCORES          # rows per core (1024)
NT = RPC // 128            # anchor tiles per core (8)
WINB = 40                  # window blocks: global blocks 8p .. 8p+39
WIN = WINB * 128           # 5120 columns
BIG = 128.0                # same-group mask magnitude (exact in fp8)
CNT_HOMO = float((B // K) * K * (K - 1))                 # 24576
CNT_HETER = float(K * K * (B // K) * (B // K - 1) // 2)  # 33538048

# per-engine accumulator tiles (separate so the tile-version write chains
# never couple the engines): raw hinge slots from a counter, then the
# specials.  accA: [0..15]=raw, 16=kacc, 17=s32.  accV: [0..16]=raw, 17=hom.
SLOT_K = 16
SLOT_S = 17
SLOT_M = 17
NSLOT = 18

_CACHE = {}


def _build_program():
    nc = bacc.Bacc("TRN2", target_bir_lowering=False, debug=False)

    xw_in = nc.dram_tensor("xw_in", [97, 2 * WIN], FP8, kind="ExternalInput").ap()
    xs_in = nc.dram_tensor("xs_in", [97, 2 * RPC], FP8, kind="ExternalInput").ap()
    mm_in = nc.dram_tensor("mm_in", [128, 1024], F32, kind="ExternalInput").ap()
    acca_out = nc.dram_tensor("acca_out", [128, NSLOT], F32, kind="ExternalOutput").ap()
    accv_out = nc.dram_tensor("accv_out", [128, NSLOT], F32, kind="ExternalOutput").ap()

    Relu = mybir.ActivationFunctionType.Relu
    ADD = mybir.AluOpType.add
    MULT = mybir.AluOpType.mult
    MAX = mybir.AluOpType.max

    xw_r = xw_in.rearrange("p (two n) -> p two n", two=2)
    xs_r = xs_in.rearrange("p (two n) -> p two n", two=2)

    with tile.TileContext(nc) as tc, ExitStack() as ctx:
        cp = ctx.enter_context(tc.tile_pool(name="cp", bufs=1))
        rp = ctx.enter_context(tc.tile_pool(name="rp", bufs=3))
        pa = ctx.enter_context(tc.tile_pool(name="pa", bufs=2, space="PSUM"))
        pv = ctx.enter_context(tc.tile_pool(name="pv", bufs=2, space="PSUM"))

        xw = cp.tile([97, 2, WIN], FP8, tag="xw")
        xs = cp.tile([97, 2, RPC], FP8, tag="xs")
        wrm = cp.tile([1, 2], mybir.dt.bfloat16, tag="wrm")
        maskm = cp.tile([128, 1024], F32, tag="maskm")
        zeros = cp.tile([128, 1024], F32, tag="zeros")
        acca = cp.tile([128, NSLOT], F32, tag="acca")
        accv = cp.tile([128, NSLOT], F32, tag="accv")
        sx = cp.tile([128, 640], F32, tag="sx")
        sx2 = cp.tile([128, 512], F32, tag="sx2")

        # xs first on the HWDGE (sync) queue — every matmul needs it; the
        # first window chunk goes on the gpsimd (SWDGE) queue in parallel.
        # maskm (f32, slow transfer) is issued at corrM emission, mid-stream.
        nc.sync.dma_start(xs[:], xs_r)
        # dummy matmul at t~0.1us starts the PE p-state ramp clock so real
        # matmuls (first at ~3.2us) run at full frequency immediately
        nc.vector.memset(wrm[:], 1.0)
        gw = pa.tile([1, 2], F32, tag="ga")
        nc.tensor.matmul(gw[:], wrm[:, 0:1], wrm[:], start=True, stop=True)
        nc.vector.memset(zeros[:], 0.0)

        def emit_prep(c):
            # 1024-col DMA chunks across THREE queues at startup: xs rides
            # sync, c0 rides gpsimd, c1 rides the idle Act sequencer, so all
            # three early tensors land in parallel ~3us; later chunks
            # alternate sync/gpsimd.
            lo = c * 1024
            eng = nc.gpsimd if c % 2 == 0 else nc.sync
            eng.dma_start(xw[:, :, lo:lo + 1024], xw_r[:, :, lo:lo + 1024])

        def emit_main(t, mc, on_act):
            # columns: window cols (t+1)*128 + mc*1024 .. +1024 (mc=3: +896)
            base = (t + 1) * 128 + mc * 1024
            width = 896 if mc == 3 else 1024
            # the final (split) unit draws from the Act-side ring, which
            # drains earlier than DVE's at the tail
            pool = pa if (on_act or (t, mc) == (7, 3)) else pv
            tag = "ga" if (on_act or (t, mc) == (7, 3)) else "gv"
            g = pool.tile([128, 1024], F32, tag=tag)
            st = xs[0:65, :, t * 128:(t + 1) * 128]
            for lo in range(0, width, 512):
                w = min(512, width - lo)
                nc.tensor.matmul(g[:, lo:lo + w], st,
                                 xw[0:65, :, base + lo:base + lo + w],
                                 start=True, stop=True, perf_mode=DR)
            if (t, mc) == (7, 3):
                # fine-balance: DVE takes cols 0:512, Act takes 512:896;
                # scratch outputs keep the halves parallel
                sv = slot_v[0]
                slot_v[0] += 1
                nc.vector.scalar_tensor_tensor(sx[:, 0:640], g[:, 0:640],
                                               0.0, zeros[:, 0:640], ADD, MAX,
                                               accum_out=accv[:, sv:sv + 1])
                sa = slot_a[0]
                slot_a[0] += 1
                nc.scalar.activation(sx2[:, 0:width - 640], g[:, 640:width],
                                     Relu, bias=0.0, scale=1.0,
                                     accum_out=acca[:, sa:sa + 1])
            elif on_act:
                sa = slot_a[0]
                slot_a[0] += 1
                nc.scalar.activation(g[:, 0:width], g[:, 0:width], Relu,
                                     bias=0.0, scale=1.0,
                                     accum_out=acca[:, sa:sa + 1])
            else:
                sv = slot_v[0]
                slot_v[0] += 1
                nc.vector.scalar_tensor_tensor(g[:, 0:width], g[:, 0:width],
                                               0.0, zeros[:, 0:width], ADD, MAX,
                                               accum_out=accv[:, sv:sv + 1])

        def emit_corr(kind):
            # batched over all 8 anchor tiles: one [128, 1024] PSUM tile,
            # 8 x 128-col matmuls; start/stop only at 512-col (bank) edges
            on_act = kind in ("H", "S")
            pool = pa if on_act else pv
            g = pool.tile([128, 1024], F32, tag="ga" if on_act else "gv")
            for t in range(NT):
                if kind == "H":        # in-block hinge, group-masked
                    st = xs[0:97, :, t * 128:(t + 1) * 128]
                    mv = xw[0:97, :, t * 128:(t + 1) * 128]
                elif kind == "S":      # distance-32 blocks
                    st = xs[0:65, :, t * 128:(t + 1) * 128]
                    mv = xw[0:65, :, (t + 32) * 128:(t + 33) * 128]
                else:                  # "M": pure Gram diag for homo
                    st = xs[0:64, :, t * 128:(t + 1) * 128]
                    mv = xw[0:64, :, t * 128:(t + 1) * 128]
                nc.tensor.matmul(g[:, t * 128:(t + 1) * 128], st, mv,
                                 start=(t % 4 == 0), stop=(t % 4 == 3),
                                 perf_mode=DR, skip_group_check=True)
            if kind == "H":
                nc.scalar.activation(g[:], g[:], Relu, bias=0.0, scale=1.0,
                                     accum_out=acca[:, SLOT_K:SLOT_K + 1])
            elif kind == "S":
                # split across engines: Act 0:512, DVE 512:1024 (scratch
                # outs keep the halves parallel)
                nc.scalar.activation(sx2[:], g[:, 0:512], Relu,
                                     bias=0.0, scale=1.0,
                                     accum_out=acca[:, SLOT_S:SLOT_S + 1])
                sv = slot_v[0]
                slot_v[0] += 1
                nc.vector.scalar_tensor_tensor(sx[:, 0:512], g[:, 512:1024],
                                               0.0, zeros[:, 0:512], ADD, MAX,
                                               accum_out=accv[:, sv:sv + 1])
            else:
                nc.vector.scalar_tensor_tensor(g[:], g[:], 0.0, maskm[:],
                                               ADD, MULT,
                                               accum_out=accv[:, SLOT_M:SLOT_M + 1])

        # items: (needed window cols, sort order, kind, t, mc).  corrH early
        # (first Act work), corrM mid-stream (its maskm DMA is issued late),
        # corrS at the end.
        items = [(1024, -1, "corrH", 0, 0), (1024, 2500, "corrM", 0, 0)]
        for t in range(NT):
            for mc in range(4):
                width = 896 if mc == 3 else 1024
                need = (t + 1) * 128 + mc * 1024 + width
                key = 1025 if (t, mc) == (7, 0) else need
                items.append((need, key, "main", t, mc))
        items.append((WIN, WIN + 1, "corrS", 0, 0))
        items.sort(key=lambda it: it[1])

        slot_a = [0]
        slot_v = [0]

        # engine split: DVE first (its first unit needs chunks 0+1 while Act
        # opens with corrH); the very last main (t=7, mc=3, 896 wide) also
        # goes to DVE so Act (which additionally has corrH + corrS) balances:
        # Act = 15 mains + H + S, DVE = 17 mains + M.
        c_done = 0
        n_main = 0
        for need, _, kind, t, mc in items:
            while c_done * 1024 < need:
                emit_prep(c_done)
                c_done += 1
            if kind == "main":
                on_act = (n_main % 2 == 1) and n_main != 31
                emit_main(t, mc, on_act=on_act)
                n_main += 1
            else:
                if kind == "corrM":
                    nc.gpsimd.dma_start(maskm[:], mm_in)
                emit_corr(kind[-1])
        while c_done * 1024 < WIN:
            emit_prep(c_done)
            c_done += 1

        nc.sync.dma_start(acca_out, acca[:])
        nc.gpsimd.dma_start(accv_out, accv[:])

    nc.compile()
    return nc


def _stage_inputs(x: np.ndarray):
    xt = np.ascontiguousarray(x.T)                      # [128, 8192] f32
    sq = (x.astype(np.float64) ** 2).sum(1)
    b = ((1.0 - sq) / 2.0).astype(np.float32)           # column bias
    c = (-sq / 2.0).astype(np.float32)                  # anchor bias

    lm = np.arange(128) // 4                            # local group ids
    gsel = (lm[None, :] == np.arange(32)[:, None])      # [32, 128] indicators

    # maskM = 6I - 2*maskh tiled 8x -> [128, 1024] f32
    same = lm[:, None] == lm[None, :]
    mM = (6.0 * np.eye(128) - 2.0 * (same & ~np.eye(128, dtype=bool))).astype(np.float32)
    mm_tile = np.ascontiguousarray(np.tile(mM, (1, 8)))

    in_maps = []
    for p in range(NCORES):
        cols = (np.arange(WIN) + p * RPC) % B
        xw8 = np.zeros((97, 2, WIN), dtype=F8NP)
        xw8[0:64, 0, :] = xt[0:64, cols].astype(F8NP)
        xw8[0:64, 1, :] = xt[64:128, cols].astype(F8NP)
        xw8[64, 0, :] = b[cols].astype(F8NP)
        xw8[64, 1, :] = np.float32(1.0)
        xw8[65:97, 0, :] = (-BIG * gsel[:, np.arange(WIN) % 128]).astype(F8NP)

        acols = cols[0:RPC]
        xs8 = np.zeros((97, 2, RPC), dtype=F8NP)
        xs8[0:64, 0, :] = xt[0:64, acols].astype(F8NP)
        xs8[0:64, 1, :] = xt[64:128, acols].astype(F8NP)
        xs8[64, 0, :] = np.float32(1.0)
        xs8[64, 1, :] = c[acols].astype(F8NP)
        xs8[65:97, 0, :] = gsel[:, np.arange(RPC) % 128].astype(F8NP)

        in_maps.append({
            "xw_in": np.ascontiguousarray(xw8.reshape(97, 2 * WIN)),
            "xs_in": np.ascontiguousarray(xs8.reshape(97, 2 * RPC)),
            "mm_in": mm_tile,
        })
    return in_maps


def kernel(x: np.ndarray):
    x = np.asarray(x, dtype=np.float32)
    assert x.shape == (B, D)

    if "nc" not in _CACHE:
        _CACHE["nc"] = _build_program()
    nc = _CACHE["nc"]

    in_maps = _stage_inputs(x)
    res = bass_utils.run_bass_kernel_spmd(nc, in_maps, core_ids=list(range(NCORES)))

    raw = 0.0
    kcc = 0.0
    s32 = 0.0
    hom = 0.0
    for p in range(NCORES):
        aa = res.results[p]["acca_out"].astype(np.float64)
        av = res.results[p]["accv_out"].astype(np.float64)
        raw += aa[:, 0:16].sum() + av[:, 0:17].sum()
        kcc += aa[:, SLOT_K].sum()
        s32 += aa[:, SLOT_S].sum()
        hom += av[:, SLOT_M].sum()

    # main units cover each cross-block unordered pair at distance 1..31 once;
    # the additive dist-32 pass covers those pairs twice; kcc covers each
    # in-block cross-group pair twice.  accumulated values are relu((1-d2)/2)
    # = relu(1-d2)/2.
    # a sum of relu terms is mathematically >= 0; clamp away engine
    # summation noise so an all-inactive hinge yields exactly 0.0
    heter_sum = max(2.0 * raw + s32 + kcc, 0.0)
    loss_homo = np.float32(hom / CNT_HOMO)
    loss_heter = np.float32(heter_sum / CNT_HETER)
    return loss_homo, loss_heter
